# revision 5
# baseline (speedup 1.0000x reference)
"""Distributed GQA attention block (dense_transformer) for 8 TRN2 NeuronCores.

Sharding: Megatron-style head sharding for QKV+attention (each core owns 4 Q
heads / 1 KV head), Ulysses-style AllToAll to switch to sequence sharding for
the output projection (each core owns 256 rows per batch).

Layouts (per core, SPMD identical graph):
  - All activations kept transposed: QT/KT [head_dim, rows] so scores come out
    as S^T [k, q] and softmax reduces over the partition axis via matmul with a
    fused ones-column in V (denominator for free).
  - RoPE handled by permuting head dims (evens|odds) in the weights on the
    host, so rotation pairs are partition halves: out = qn*cos + swap(qn)*sin±.
  - RMSNorm partition-reduction via indicator matmul; rsqrt via ACT ln/exp
    (same ACT table set as softmax exp -> zero table switches).
Compute in bf16 on the TensorEngine (f32 accumulation), f32 softmax.
"""
import sys

if '/opt/trn_rl_repo' not in sys.path:
    sys.path.insert(0, '/opt/trn_rl_repo')

import numpy as np
import ml_dtypes

N_CORES = 8
B, S, D = 2, 2048, 2048
DH = 64
HLOC = 4            # Q heads per core
ROWS = B * S        # 4096
NKT = D // 128      # 16 contraction tiles
RC = 512            # row chunk
NCHUNK = ROWS // RC # 8
EPS = 1e-6

_cache = {}


def _build():
    import concourse.mybir as mybir
    import concourse.tile as tile
    from concourse import bacc
    from concourse.bass import ts, ds

    f32 = mybir.dt.float32
    bf = mybir.dt.bfloat16
    AF = mybir.ActivationFunctionType
    MUL = mybir.AluOpType.mult

    nc = bacc.Bacc()
    x_sb = nc.declare_dram_parameter("x_sb", [128, NKT * ROWS], bf, isOutput=False)
    wq_sb = nc.declare_dram_parameter("wq_sb", [128, NKT * 2 * 128], bf, isOutput=False)
    wk_sb = nc.declare_dram_parameter("wk_sb", [128, NKT * 128], bf, isOutput=False)
    wv_sb = nc.declare_dram_parameter("wv_sb", [128, NKT * 64], bf, isOutput=False)
    wo_sb = nc.declare_dram_parameter("wo_sb", [128, 16 * 16 * 128], bf, isOutput=False)
    cos_sb = nc.declare_dram_parameter("cos_sb", [128, ROWS], f32, isOutput=False)
    sin_sb = nc.declare_dram_parameter("sin_sb", [128, ROWS], f32, isOutput=False)
    msk_sb = nc.declare_dram_parameter("msk_sb", [128, 896], bf, isOutput=False)
    ind_sb = nc.declare_dram_parameter("ind_sb", [128, 2], bf, isOutput=False)
    idn_sb = nc.declare_dram_parameter("idn_sb", [64, 64], bf, isOutput=False)
    sc_sb = nc.declare_dram_parameter("sc_sb", [64, 2], f32, isOutput=False)
    out_ext = nc.declare_dram_parameter("out", [D, 512], f32, isOutput=True)

    with tile.TileContext(nc) as tc:
        with (
            tc.tile_pool(name="cp", bufs=1) as cp,
            tc.tile_pool(name="xp", bufs=2) as xp,
            tc.tile_pool(name="wp", bufs=3) as wp,
            tc.tile_pool(name="sp", bufs=2) as sp,
            tc.tile_pool(name="ep", bufs=4) as ep,
            tc.tile_pool(name="dram", bufs=1, space="DRAM") as dram,
            tc.tile_pool(name="pproj", bufs=2, space="PSUM") as pproj,
            tc.tile_pool(name="pscore", bufs=2, space="PSUM") as pscore,
            tc.tile_pool(name="po", bufs=2, space="PSUM") as po,
            tc.tile_pool(name="psm", bufs=2, space="PSUM") as psm,
        ):
            # ---- persistent constants ----
            wq = cp.tile([128, NKT * 2 * 128], bf)
            for i in range(4):
                nc.sync.dma_start(wq[:, ds(i * 1024, 1024)], wq_sb[:, ds(i * 1024, 1024)])
            wkt = cp.tile([128, NKT * 128], bf)
            nc.sync.dma_start(wkt[:], wk_sb[:])
            wvt = cp.tile([128, NKT * 64], bf)
            nc.sync.dma_start(wvt[:], wv_sb[:])
            msk = cp.tile([128, 896], bf)
            nc.sync.dma_start(msk[:], msk_sb[:])
            ind = cp.tile([128, 2], bf)
            nc.sync.dma_start(ind[:], ind_sb[:])
            idn = cp.tile([64, 64], bf)
            nc.sync.dma_start(idn[:], idn_sb[:])
            sc2 = cp.tile([64, 2], f32)
            nc.sync.dma_start(sc2[:], sc_sb[:])
            epsc = cp.tile([1, 1], f32)
            nc.gpsimd.memset(epsc[:], EPS)

            QTn = [cp.tile([128, ROWS], bf, name=f"qtn{i}") for i in range(2)]
            KTd = [cp.tile([128, S], bf, name=f"ktd{b}") for b in range(B)]
            Vb1 = [cp.tile([128, 16 * 65], bf, name=f"vb{b}") for b in range(B)]
            attb = cp.tile([128, 16 * 256], bf)

            a2a_in = [dram.tile([2048, 256], f32, name=f"a2ain{b}") for b in range(B)]
            a2a_out = [dram.tile([2048, 256], f32, name=f"a2aout{b}") for b in range(B)]

            # ---- norm + rope on a projection psum tile ----
            def norm_rope(ps, dst_ap, cosc, sinc, sc_col, dup):
                sq = sp.tile([128, RC], bf, tag="sq")
                nc.scalar.activation(sq[:], ps[:], AF.Square)
                rbcs = []
                for h in range(1 if dup else 2):
                    ssp = psm.tile([1, RC], f32, tag="small", name=f"ss{h}")
                    nc.tensor.matmul(ssp[:], ind[:, h:h + 1], sq[:], start=True, stop=True)
                    lg = sp.tile([1, RC], f32, tag="lg", bufs=4)
                    nc.scalar.activation(lg[:], ssp[:], AF.Ln, scale=1.0 / 64, bias=epsc[:])
                    rstd = sp.tile([1, RC], f32, tag="rstd", bufs=4)
                    nc.scalar.activation(rstd[:], lg[:], AF.Exp, scale=-0.5)
                    rbc = sp.tile([64, RC], f32, tag="rbc", bufs=4, name=f"rbc{h}")
                    nc.gpsimd.partition_broadcast(rbc[:], rstd[:])
                    rbcs.append(rbc)
                if dup:
                    rbcs.append(rbcs[0])
                qn = sp.tile([128, RC], f32, tag="qn")
                nc.vector.scalar_tensor_tensor(
                    qn[0:64, :], ps[0:64, :], sc2[:, sc_col:sc_col + 1], rbcs[0][:], MUL, MUL)
                nc.vector.scalar_tensor_tensor(
                    qn[64:128, :], ps[64:128, :], sc2[:, sc_col:sc_col + 1], rbcs[1][:], MUL, MUL)
                swp = sp.tile([128, RC], f32, tag="swp")
                for g in range(4):
                    nc.sync.dma_start(swp[ts(g, 32), :], qn[ts(g ^ 1, 32), :])
                nc.vector.tensor_mul(qn[:], qn[:], cosc[:])
                nc.vector.tensor_mul(swp[:], swp[:], sinc[:])
                nc.vector.tensor_add(dst_ap, qn[:], swp[:])

            # ---- one row-chunk of projections ----
            def proj_chunk(r):
                b, sl = r // 4, r % 4
                xt = xp.tile([128, NKT, RC], bf, tag="xt")
                for k in range(NKT):
                    nc.sync.dma_start(xt[:, k, :], x_sb[:, ds(k * ROWS + r * RC, RC)])
                cosc = sp.tile([128, RC], f32, tag="cos")
                nc.sync.dma_start(cosc[:], cos_sb[:, ds(r * RC, RC)])
                sinc = sp.tile([128, RC], f32, tag="sin")
                nc.sync.dma_start(sinc[:], sin_sb[:, ds(r * RC, RC)])
                for hp in range(2):
                    psq = pproj.tile([128, RC], f32, tag="proj", name=f"psq{hp}")
                    for k in range(NKT):
                        nc.tensor.matmul(psq[:], wq[:, ds((k * 2 + hp) * 128, 128)],
                                         xt[:, k, :], start=(k == 0), stop=(k == NKT - 1))
                    norm_rope(psq, QTn[hp][:, ds(r * RC, RC)], cosc, sinc, 0, False)
                psk = pproj.tile([128, RC], f32, tag="proj")
                for k in range(NKT):
                    nc.tensor.matmul(psk[:], wkt[:, ts(k, 128)], xt[:, k, :],
                                     start=(k == 0), stop=(k == NKT - 1))
                norm_rope(psk, KTd[b][:, ds(sl * RC, RC)], cosc, sinc, 1, True)
                psv = pproj.tile([64, RC], f32, tag="proj")
                for k in range(NKT):
                    nc.tensor.matmul(psv[:], wvt[:, ts(k, 64)], xt[:, k, :],
                                     start=(k == 0), stop=(k == NKT - 1))
                vtmp = sp.tile([64, RC], bf, tag="vtmp")
                nc.vector.tensor_copy(vtmp[:], psv[:])
                for t4 in range(4):
                    tp = psm.tile([128, 64], bf, tag="small", name="tp")
                    nc.tensor.transpose(tp[:], vtmp[:, ts(t4, 128)], idn[:])
                    gt = sl * 4 + t4
                    nc.vector.tensor_copy(Vb1[b][:, ds(gt * 65, 64)], tp[:])
                    nc.gpsimd.memset(Vb1[b][:, ds(gt * 65 + 64, 1)], 1.0)

            # ---- one attention block: batch b, head-pair hp, q-slice qs ----
            def attn_block(b, hp, qs):
                psO = [po.tile([65, RC], f32, tag="o", name=f"psO{t}") for t in range(2)]
                jmax = qs * 4 + 3
                for j in range(jmax + 1):
                    dj = j - qs * 4
                    p = dj * 128 if dj >= 0 else 0
                    N = RC - p
                    qb = b * S + qs * RC + p
                    psS = [pscore.tile([128, RC], f32, tag="s", name=f"psS{t}")
                           for t in range(2)]
                    nc.tensor.matmul(psS[0][:, 0:N], KTd[b][0:64, ts(j, 128)],
                                     QTn[hp][0:64, ds(qb, N)], start=True, stop=True,
                                     tile_position=(0, 0))
                    nc.tensor.matmul(psS[1][:, 0:N], KTd[b][64:128, ts(j, 128)],
                                     QTn[hp][64:128, ds(qb, N)], start=True, stop=True,
                                     tile_position=(64, 0))
                    for t in range(2):
                        E = ep.tile([128, RC], bf, tag="E")
                        if p:
                            nc.vector.memset(E[:, 0:p], 0.0)
                        nc.scalar.activation(E[:, p:RC], psS[t][:, 0:N], AF.Exp, scale=0.125)
                        if dj >= 0:
                            nc.vector.tensor_mul(E[:, ds(p, 128)], E[:, ds(p, 128)],
                                                 msk[:, ds(384, 128)])
                        nc.tensor.matmul(psO[t][:], Vb1[b][:, ds(j * 65, 65)], E[:],
                                         start=(j == 0), stop=(j == jmax))
                for t in range(2):
                    hl = 2 * hp + t
                    recip = sp.tile([1, RC], f32, tag="recip")
                    nc.vector.reciprocal(recip[:], psO[t][64:65, :])
                    rbco = sp.tile([64, RC], f32, tag="rbco")
                    nc.gpsimd.partition_broadcast(rbco[:], recip[:])
                    on = sp.tile([64, RC], f32, tag="on")
                    nc.vector.tensor_mul(on[:], psO[t][0:64, :], rbco[:])
                    nc.sync.dma_start(
                        a2a_in[b][ds(256 * (2 * qs) + hl * 64, 64), :], on[:, 0:256])
                    nc.sync.dma_start(
                        a2a_in[b][ds(256 * (2 * qs + 1) + hl * 64, 64), :], on[:, 256:512])

            def do_a2a(b):
                nc.gpsimd.collective_compute(
                    "AllToAll", mybir.AluOpType.bypass,
                    replica_groups=[list(range(N_CORES))],
                    ins=[a2a_in[b].opt()], outs=[a2a_out[b].opt()])

            def oproj(b):
                for k in range(16):
                    attf = sp.tile([128, 256], f32, tag="attf")
                    nc.sync.dma_start(attf[:], a2a_out[b][ts(k, 128), :])
                    nc.vector.tensor_copy(attb[:, ts(k, 256)], attf[:])
                for m in range(16):
                    wostrip = wp.tile([128, 2048], bf, tag="wo")
                    nc.sync.dma_start(wostrip[:], wo_sb[:, ds(m * 2048, 2048)])
                    psf = pproj.tile([128, 256], f32, tag="proj", name="psf")
                    for k in range(16):
                        nc.tensor.matmul(psf[:], wostrip[:, ts(k, 128)], attb[:, ts(k, 256)],
                                         start=(k == 0), stop=(k == 15))
                    ofin = sp.tile([128, 256], f32, tag="ofin")
                    nc.vector.tensor_copy(ofin[:], psf[:])
                    nc.sync.dma_start(out_ext[ts(m, 128), ds(b * 256, 256)], ofin[:])

            # ---- emission schedule ----
            for r in range(4):
                proj_chunk(r)
            blocks = [(hp, qs) for qs in range(4) for hp in range(2)]
            for i, (hp, qs) in enumerate(blocks):
                attn_block(0, hp, qs)
                if i in (0, 2, 4, 6):
                    proj_chunk(4 + i // 2)
            do_a2a(0)
            for i, (hp, qs) in enumerate(blocks):
                attn_block(1, hp, qs)
                if i == 1:
                    oproj(0)
            do_a2a(1)
            oproj(1)

    nc.compile()
    return nc


def _host_prep(x, freqs_cos, freqs_sin, wq, wk, wv, wo, q_scale, k_scale):
    bfd = ml_dtypes.bfloat16
    perm = np.concatenate([np.arange(0, 64, 2), np.arange(1, 64, 2)])

    xT = np.ascontiguousarray(x.reshape(ROWS, D).T)
    x_sb = np.ascontiguousarray(
        xT.reshape(NKT, 128, ROWS).transpose(1, 0, 2).reshape(128, NKT * ROWS)
    ).astype(bfd)

    ct = np.concatenate([freqs_cos.T, freqs_cos.T], axis=1)   # [32, 4096]
    st = np.concatenate([freqs_sin.T, freqs_sin.T], axis=1)
    cos_sb = np.ascontiguousarray(np.tile(ct, (4, 1))).astype(np.float32)
    sin_sb = np.ascontiguousarray(np.concatenate([-st, st, -st, st], 0)).astype(np.float32)

    r = np.arange(128)[:, None]
    c = np.arange(896)[None, :]
    msk_sb = (c >= r + 384).astype(bfd)
    ind_sb = np.zeros((128, 2), bfd)
    ind_sb[0:64, 0] = 1
    ind_sb[64:128, 1] = 1
    idn_sb = np.eye(64, dtype=bfd)
    sc_sb = np.stack([q_scale[perm], k_scale[perm]], axis=1).astype(np.float32)

    woT = wo.T.astype(np.float32)  # [hdim, dout]
    wo_sb = np.ascontiguousarray(
        woT.reshape(16, 128, 16, 128).transpose(1, 2, 0, 3).reshape(128, 16 * 16 * 128)
    ).astype(bfd)

    shared = dict(x_sb=x_sb, cos_sb=cos_sb, sin_sb=sin_sb, msk_sb=msk_sb,
                  ind_sb=ind_sb, idn_sb=idn_sb, sc_sb=sc_sb, wo_sb=wo_sb)

    in_maps = []
    for cc in range(N_CORES):
        wq_c = wq[cc * 256:(cc + 1) * 256].reshape(4, 64, D)[:, perm].reshape(256, D)
        wqT = wq_c.T  # [D, 256]
        wq_core = np.ascontiguousarray(
            wqT.reshape(NKT, 128, 2, 128).transpose(1, 0, 2, 3).reshape(128, NKT * 256)
        ).astype(bfd)
        wk_c = wk[cc * 64:(cc + 1) * 64][perm]
        wkTd = np.concatenate([wk_c, wk_c], 0).T  # [D, 128]
        wk_core = np.ascontiguousarray(
            wkTd.reshape(NKT, 128, 128).transpose(1, 0, 2).reshape(128, NKT * 128)
        ).astype(bfd)
        wvT = wv[cc * 64:(cc + 1) * 64].T  # [D, 64]
        wv_core = np.ascontiguousarray(
            wvT.reshape(NKT, 128, 64).transpose(1, 0, 2).reshape(128, NKT * 64)
        ).astype(bfd)
        in_maps.append(dict(shared, wq_sb=wq_core, wk_sb=wk_core, wv_sb=wv_core))
    return in_maps


def kernel(x, freqs_cos, freqs_sin, wq, wk, wv, wo, q_scale, k_scale, _trace=False):
    from concourse.bass_utils import run_bass_kernel_spmd

    if "nc" not in _cache:
        _cache["nc"] = _build()
    nc = _cache["nc"]

    args = [np.asarray(a, dtype=np.float32) for a in
            (x, freqs_cos, freqs_sin, wq, wk, wv, wo, q_scale, k_scale)]
    in_maps = _host_prep(*args)
    res = run_bass_kernel_spmd(nc, in_maps, list(range(N_CORES)), trace=_trace)
    out = np.zeros((B, S, D), np.float32)
    for cc in range(N_CORES):
        oc = res.results[cc]["out"]  # [2048, 512]
        for b in range(B):
            out[b, 256 * cc:256 * (cc + 1), :] = oc[:, 256 * b:256 * (b + 1)].T
    if _trace:
        return out, res
    return out
